# revision 28
# baseline (speedup 1.0000x reference)
"""BasicTransformerBlock on 8 TRN2 NeuronCores.

Sharding: sequence-parallel, zero collectives. The [B=2, N=2048, D=512]
residual stream is split into 8 row-blocks of 512 (4 cores per batch
element). Every core recomputes the cheap batch-wide work it needs
(adaln1 + K/V projections over its batch's 2048 rows, cond K/V), and does
attention / FFN only for its own 512 query rows.

Per-core inputs are pre-rotated with np.roll so that "own" rows are always
rows 0:512 -- the SPMD program is identical on all cores, only data differs.
Attention is permutation-invariant over keys, so rolled K/V is fine.

Layouts (SBUF tiles are [128 partitions, ...]):
  hT  = normed activations, transposed: [128 p=d%128, 4 dtile, rows] bf16
  kT  = [128 p=dout%128, 4 dtile, rows] bf16   (head pair 2t,2t+1 stacked
        in partitions 0:64 / 64:128 of dtile t)
  qz  = [128, 4 ht, 2 slot, rows] bf16: slot s holds head 2ht+s in its own
        64-partition half, the other half ZERO.
  vC  = [128 p=row%128, jt, 4 ht, 193] bf16 overlapped pair tile:
        [v_even(64) | 1 | v_odd(64) | 1 | v_even[0:63]]; head 2ht+s uses
        columns 65s:65s+128. Column 64/129 of each window is 1.0, which
        makes the attention-weight row-sum (softmax denominator) fall out
        of the same matmul that computes attn@v.

Every attention matmul is a full 128x128-array op (scores: K=128 via the
stacked head pair against a zero-padded q half; attn@v: M=128 via the
overlapped v windows). The PE HAM activity monitor only counts full-array
matmuls as "busy": half-array ops (K=64 or M=65) leave the clock gate at
K=4/8 (1.2 GHz effective) for the whole attention phase, which is where
the previous version lost ~120us.

Scores are computed transposed, sT[j, i], so exp() runs on ScalarE straight
out of PSUM and softmax normalization is applied per head on the tiny
attn@v result. Matmul operands are bf16 (weights are cast during the DMA
load by SWDGE); all accumulation/psum/residual math stays fp32.
"""

import contextlib

import numpy as np

import concourse.bass as bass
import concourse.mybir as mybir
import concourse.tile as tile
from concourse import bacc
from concourse.bass_utils import run_bass_kernel_spmd
from concourse.masks import make_identity

dt = mybir.dt
AF = mybir.ActivationFunctionType
OP = mybir.AluOpType

B, N, D = 2, 2048, 512
NCTX = 1024          # cond length
H = 8                # heads
HD = D // H          # 64
EPS = 1e-5
P = 128              # partitions
NCORES = 8
ROWS = 512           # own rows per core
NB = N               # batch rows per core (2048)
SCALE = HD ** -0.5   # 0.125

f32 = dt.float32
bf16 = dt.bfloat16

_CACHED = {}


def _adaln_stats(nc, stat_pool, src_tiles, n_tiles, eps_sb, chunk=4):
    """bn_stats/aggr + rstd/nmr for n_tiles row-tiles. Returns (rstd_all, nmr_all)."""
    mv_all = stat_pool.tile([P, n_tiles, 2], f32)
    rstd_all = stat_pool.tile([P, n_tiles], f32)
    nmr_all = stat_pool.tile([P, n_tiles], f32)
    for c0 in range(0, n_tiles, chunk):
        for it in range(c0, c0 + chunk):
            stats = stat_pool.tile([P, 6], f32, tag="stats")
            nc.vector.bn_stats(stats, src_tiles(it))
            nc.vector.bn_aggr(mv_all[:, it, :], stats)
        cs = slice(c0, c0 + chunk)
        nc.scalar.activation(rstd_all[:, cs], mv_all[:, cs, 1], AF.Sqrt,
                             bias=eps_sb, scale=1.0)
        nc.vector.reciprocal(rstd_all[:, cs], rstd_all[:, cs])
        nc.vector.scalar_tensor_tensor(
            nmr_all[:, cs], mv_all[:, cs, 0], -1.0, rstd_all[:, cs],
            op0=OP.mult, op1=OP.mult,
        )
    return rstd_all, nmr_all


def _adaln_apply(nc, tc, src_tiles, n_tiles, ab, rstd_all, nmr_all, hT,
                 ident_bf16, name):
    """xn = (x-mean)*rstd -> PE transpose -> fused (1+scale)/shift copy."""
    with contextlib.ExitStack() as actx:
        xn_pool = actx.enter_context(tc.tile_pool(name=f"{name}_xn", bufs=3))
        pst_pool = actx.enter_context(
            tc.tile_pool(name=f"{name}_pst", bufs=2, space="PSUM")
        )
        for it in range(n_tiles):
            xn = xn_pool.tile([P, 512], bf16, tag="xn")
            nc.scalar.activation(xn, src_tiles(it), AF.Identity,
                                 bias=nmr_all[:, it:it + 1],
                                 scale=rstd_all[:, it:it + 1])
            xnt = pst_pool.tile([P, 4, P], bf16, tag="xnt")
            for b in range(4):
                nc.tensor.transpose(
                    xnt[:, b, :], xn[:, b * P:(b + 1) * P], ident_bf16
                )
            # (1+scale)*xnt + shift on ScalarE -- DVE is the busier engine
            # in every adaln window
            for b in range(4):
                nc.scalar.activation(
                    hT[:, b, it * P:(it + 1) * P], xnt[:, b, :], AF.Identity,
                    bias=ab[:, 4 + b:5 + b], scale=ab[:, b:b + 1],
                )


def _adaln_to_hT(nc, tc, src_tiles, n_tiles, ab, hT, ident_bf16, eps_sb, name):
    with contextlib.ExitStack() as actx:
        stat_pool = actx.enter_context(tc.tile_pool(name=f"{name}_stat", bufs=4))
        rstd_all, nmr_all = _adaln_stats(nc, stat_pool, src_tiles, n_tiles, eps_sb)
        _adaln_apply(nc, tc, src_tiles, n_tiles, ab, rstd_all, nmr_all, hT,
                     ident_bf16, name)


VW = HD + 1       # 65: per-head v window [v_h(64) | 1]
VROW = VW * H     # 520: all 8 head windows of one key tile, contiguous


def _vc_tile(nc, act, pool_tag, njt):
    """Flat v tile: njt contiguous 520-col key-tile rows + 63 tail filler.
    Head h's av weight window is cols [jt*520+65h : +128] = [v_h | 1 |
    v_{h+1}[0:63]] -- M=128 without any extra copies (cols past 65 of a
    window produce garbage psum rows that are never read)."""
    vC = act.tile([P, njt * VROW + HD - 1], bf16, tag=pool_tag)
    nc.vector.memset(
        vC[:, 0:njt * VROW].rearrange("p (j h w) -> p j h w", j=njt, h=H)[:, :, :, HD:VW],
        1.0,
    )
    nc.vector.memset(vC[:, njt * VROW:], 0.0)
    return vC


def _vc_copy(nc, vC, jt, ps):
    dst = vC[:, jt * VROW:(jt + 1) * VROW].rearrange("p (h w) -> p h w", h=H)
    nc.vector.tensor_copy(dst[:, :, 0:HD], ps.rearrange("p (h d) -> p h d", h=H))


def _flush_av(nc, vC, njt, pend, avps, dn_pool, dnms):
    ht, et, jt = pend
    for s in range(2):
        h = 2 * ht + s
        nc.tensor.matmul(
            avps[h], vC[:, jt * VROW + VW * h:jt * VROW + VW * h + P], et[:, s, :],
            start=(jt == 0), stop=(jt == njt - 1),
        )
    if jt == njt - 1:
        # pair finished: pull the denominator rows out so psum banks free
        for s in range(2):
            h = 2 * ht + s
            dnm_h = dn_pool.tile([1, 512], bf16, tag="dnm")
            nc.vector.tensor_copy(dnm_h, avps[h][HD:HD + 1, :])
            dnms[h] = dnm_h


def _attention(nc, tc, act, qz, kT, vC, njt, wo, ob_row, ones_row,
               x_res, x_out, name):
    """Transposed-score attention for 8 heads over own 512 rows.

    qz: [128, 4 ht, 2, rows]; kT: [128, 4, keys]; vC: [128, njt, 4, 193].
    All attention matmuls are full 128x128-array (keeps the HAM clock
    gate open). Writes x_out = attn_out @ wo + ob + x_res.
    """
    av_all = act.tile([P, 4, ROWS], bf16, tag="tH")
    with (
        tc.tile_pool(name=f"{name}_ps_s", bufs=2, space="PSUM") as ps_s,
        tc.tile_pool(name=f"{name}_ps_av", bufs=4, space="PSUM") as ps_av,
        tc.tile_pool(name=f"{name}_et", bufs=3) as et_pool,
        tc.tile_pool(name=f"{name}_dn", bufs=4) as dn_pool,
    ):
        for grp in range(2):
            heads = range(grp * 4, grp * 4 + 4)
            avps = {}
            dnms = {}
            # software-pipelined: scores/exp for step n+1 issue before the
            # av matmuls of step n, so the in-order PE stream never waits on
            # ScalarE's exp latency. The skew also crosses pair boundaries.
            pend = None   # (ht, et, jt)
            for ht in (grp * 2, grp * 2 + 1):
                for s in range(2):
                    avp = ps_av.tile([P, ROWS], f32, tag="av")
                    avps[2 * ht + s] = avp
                for jt in range(njt):
                    sps = ps_s.tile([P, 2, ROWS], f32, tag="s")
                    for s in range(2):
                        nc.tensor.matmul(
                            sps[:, s, :],
                            kT[:, ht, jt * P:(jt + 1) * P],
                            qz[:, ht, s, :],
                            start=True, stop=True,
                        )
                    et = et_pool.tile([P, 2, ROWS], bf16, tag="et")
                    nc.scalar.activation(et, sps, AF.Exp, scale=SCALE)
                    if pend is not None:
                        _flush_av(nc, vC, njt, pend, avps, dn_pool, dnms)
                    pend = (ht, et, jt)
                # flush at pair end is deferred; pend carries over
            if pend is not None:
                _flush_av(nc, vC, njt, pend, avps, dn_pool, dnms)
            # broadcast denom rows across 64 partitions via K=1 matmuls,
            # then one full-width reciprocal per head-pair (psum -> sbuf)
            rbs = {}
            for pair in range(2):
                rb = ps_s.tile([P, 2, ROWS], f32, tag="s")
                for sub in range(2):
                    hh = grp * 4 + pair * 2 + sub
                    nc.tensor.matmul(
                        rb[sub * HD:(sub + 1) * HD, 0, :],
                        ones_row[0:1, 0:HD],
                        dnms[hh][0:1, :],
                        start=True, stop=True,
                    )
                rb_sb = et_pool.tile([P, ROWS], bf16, tag="rb")
                with nc.allow_low_precision(reason="bf16 softmax recip"):
                    nc.vector.reciprocal(rb_sb, rb[:, 0, :])
                rbs[pair] = rb_sb
            for h in heads:
                po = 64 * (h % 2)
                nc.vector.scalar_tensor_tensor(
                    av_all[po:po + HD, h // 2, :],
                    avps[h][0:HD, :], 1.0, rbs[(h % 4) // 2][po:po + HD, :],
                    op0=OP.mult, op1=OP.mult,
                )
    # out-projection + bias + residual
    with tc.tile_pool(name=f"{name}_ps_o", bufs=2, space="PSUM") as ps_o:
        for it in range(4):
            ps = ps_o.tile([P, D], f32, tag="o")
            for dt_ in range(4):
                nc.tensor.matmul(
                    ps, av_all[:, dt_, it * P:(it + 1) * P], wo[:, dt_, :],
                    start=(dt_ == 0), stop=False,
                )
            nc.tensor.matmul(
                ps, ones_row[0:1, 0:P], ob_row, start=False, stop=True,
            )
            nc.vector.tensor_tensor(x_out[:, it, :], ps, x_res[:, it, :], op=OP.add)


def build(max_phase=5):
    nc = bacc.Bacc(None, target_bir_lowering=False)

    # ---------------- I/O ----------------
    xb = nc.dram_tensor("xb", [NB, D], f32, kind="ExternalInput")
    condb = nc.dram_tensor("condb", [NCTX, D], f32, kind="ExternalInput")
    t_in = nc.dram_tensor("t", [D], f32, kind="ExternalInput")
    nw = {}
    nb_ = {}
    for l in (1, 2, 4):
        nw[l] = nc.dram_tensor(f"n{l}_w", [D, 2 * D], f32, kind="ExternalInput")
        nb_[l] = nc.dram_tensor(f"n{l}_b", [2 * D], f32, kind="ExternalInput")
    aw = {}
    for a in (1, 2):
        for w in "qkvo":
            aw[a, w] = nc.dram_tensor(f"a{a}_{w}", [D, D], f32, kind="ExternalInput")
        aw[a, "ob"] = nc.dram_tensor(f"a{a}_ob", [D], f32, kind="ExternalInput")
    ff_w1 = nc.dram_tensor("ff_w1", [D, 8 * D], f32, kind="ExternalInput")
    ff_b1 = nc.dram_tensor("ff_b1", [8 * D], f32, kind="ExternalInput")
    ff_w2 = nc.dram_tensor("ff_w2", [4 * D, D], f32, kind="ExternalInput")
    ff_b2 = nc.dram_tensor("ff_b2", [D], f32, kind="ExternalInput")
    out = nc.dram_tensor("out", [ROWS, D], f32, kind="ExternalOutput")

    with tile.TileContext(nc) as tc, contextlib.ExitStack() as ctx:
        const = ctx.enter_context(tc.tile_pool(name="const", bufs=1))
        wpool = ctx.enter_context(tc.tile_pool(name="wpool", bufs=1))
        act = ctx.enter_context(tc.tile_pool(name="act", bufs=1))

        ident_bf16 = const.tile([P, P], bf16)
        make_identity(nc, ident_bf16)
        ident_f32 = const.tile([P, P], f32)
        make_identity(nc, ident_f32)
        ones_row = const.tile([1, P], bf16)
        nc.vector.memset(ones_row, 1.0)
        eps_sb = const.tile([P, 1], f32)
        nc.vector.memset(eps_sb, EPS)

        # PE warmup: ~50 dependency-free matmuls fill the otherwise idle
        # startup window and lift the HAM clock gate to 2.4 GHz early
        with tc.tile_pool(name="warm", bufs=1, space="PSUM") as warm_pool:
            wps = warm_pool.tile([P, P], f32)
            for _ in range(50):
                nc.tensor.matmul(wps, ident_bf16, ident_bf16,
                                 start=True, stop=True)

        # t as column tiles [128, 4] bf16 for emb matmul lhsT
        tT = const.tile([P, 4], bf16)
        nc.gpsimd.dma_start(tT, t_in[:].rearrange("(k p) -> p k", p=P))

        h1T = act.tile([P, 4, NB], bf16, tag="tA")
        own_x = act.tile([P, 4, D], f32, tag="tE")
        # non-own rows only feed adaln1 -> K/V; bf16 is plenty and halves
        # their SBUF footprint (the DMA casts f32->bf16 in flight)
        xrest = act.tile([P, 12, D], bf16, tag="tX")
        x_tiles = {}
        for it in range(16):
            if it < 4:
                dst = own_x[:, it, :]
                nc.sync.dma_start(dst, xb[:][it * P:(it + 1) * P, :])
            else:
                dst = xrest[:, it - 4, :]
                nc.gpsimd.dma_start(dst, xb[:][it * P:(it + 1) * P, :])
            x_tiles[it] = dst

        # adaln1 stats issue first: independent of norm weights, keeps DVE
        # busy while the emb chain waits on its weight DMAs
        n1_stat = ctx.enter_context(tc.tile_pool(name="n1_stat", bufs=4))
        if max_phase >= 1:
            rstd1, nmr1 = _adaln_stats(nc, n1_stat, lambda it: x_tiles[it],
                                       16, eps_sb)

        # ---------------- norm scale/shift params ----------------
        # emb = t @ nw + nb  -> [1, 1024] -> [128, 8] columns. Only layer 1
        # is needed early; layers 2/4 are deferred past the phase-2 issue so
        # their 2.1MB weight DMAs queue behind the attention weights.
        def _emb(l):
            with (
                tc.tile_pool(name=f"nwp{l}", bufs=1) as nwp,
                tc.tile_pool(name=f"embp{l}", bufs=1) as embp,
                tc.tile_pool(name=f"ps_emb{l}", bufs=2, space="PSUM") as ps_emb,
            ):
                nw_sb = nwp.tile([P, 4, 2 * D], bf16, tag="nw")
                nc.gpsimd.dma_start(
                    nw_sb, nw[l][:].rearrange("(k p) n -> p k n", p=P)
                )
                nb_row = embp.tile([1, 2 * D], f32, tag="nbrow")
                nc.sync.dma_start(nb_row, nb_[l][:].rearrange("(a n) -> a n", a=1))
                emb_ps = ps_emb.tile([1, 2 * D], f32, tag="embps")
                for half in range(2):
                    for kt in range(4):
                        nc.tensor.matmul(
                            emb_ps[:, half * D:(half + 1) * D],
                            tT[:, kt:kt + 1],
                            nw_sb[:, kt, half * D:(half + 1) * D],
                            start=(kt == 0), stop=(kt == 3),
                        )
                emb_row = embp.tile([1, 2 * D], f32, tag="embrow")
                nc.vector.tensor_tensor(emb_row, emb_ps, nb_row, op=OP.add)
                # row -> per-partition columns via tiny PE transposes;
                # scale columns (0:4) get the +1 fused into the psum copy
                ab_l = const.tile([P, 8], f32, tag=f"ab{l}")
                for col in range(8):
                    tp = ps_emb.tile([P, 1], f32, tag="embT")
                    nc.tensor.transpose(
                        tp, emb_row[0:1, col * P:(col + 1) * P],
                        ident_f32[0:1, 0:1]
                    )
                    nc.vector.tensor_scalar(
                        ab_l[:, col:col + 1], tp,
                        1.0 if col < 4 else 0.0, None, op0=OP.add,
                    )
            return ab_l

        ab = {1: _emb(1)}



        # ---------------- attention weights (bf16 via DMA cast) ----------
        # a1 stack shares addresses with ff_w1, a2 stack with ff_w2
        # (sequential lifetimes; Tile inserts the WAR deps).
        a_sb = {}
        for a, wtag in ((1, "wbig1"), (2, "wbig2")):
            stack = wpool.tile([P, 4, 4, D], bf16, tag=wtag)
            for wi, w in enumerate("qkvo"):
                nc.gpsimd.dma_start(
                    stack[:, :, wi, :],
                    aw[a, w][:].rearrange("(k p) n -> p k n", p=P),
                )
                a_sb[a, w] = stack[:, :, wi, :]
            ob = wpool.tile([1, D], bf16, tag=f"a{a}ob")
            nc.gpsimd.dma_start(ob, aw[a, "ob"][:].rearrange("(a n) -> a n", a=1))
            a_sb[a, "ob"] = ob


        # ---------------- phase 1: adaln1 apply -> h1T -------------------
        if max_phase < 1:
            final = own_x
        else:
            _adaln_apply(nc, tc, lambda it: x_tiles[it], 16, ab[1], rstd1,
                         nmr1, h1T, ident_bf16, "n1")
            final = own_x

        # ---------------- phase 2: projections k1T, vC1, q1z -------------
        if max_phase >= 2:
            k1T = act.tile([P, 4, NB], bf16, tag="tB")
            vC1 = _vc_tile(nc, act, "tC", 16)
            q1z = act.tile([P, 4, 2, ROWS], bf16, tag="tD")
            nc.vector.memset(q1z[HD:P, :, 0, :], 0.0)
            nc.vector.memset(q1z[0:HD, :, 1, :], 0.0)
            with tc.tile_pool(name="ps_proj1", bufs=4, space="PSUM") as ps_proj:
                for dt_ in range(4):
                    for jc in range(4):
                        ps = ps_proj.tile([P, 512], f32, tag="proj")
                        for kt in range(4):
                            nc.tensor.matmul(
                                ps,
                                a_sb[1, "k"][:, kt, dt_ * P:(dt_ + 1) * P],
                                h1T[:, kt, jc * 512:(jc + 1) * 512],
                                start=(kt == 0), stop=(kt == 3),
                            )
                        nc.scalar.copy(
                            k1T[:, dt_, jc * 512:(jc + 1) * 512], ps
                        )
                for jt in range(16):
                    ps = ps_proj.tile([P, 512], f32, tag="proj")
                    for kt in range(4):
                        nc.tensor.matmul(
                            ps,
                            h1T[:, kt, jt * P:(jt + 1) * P],
                            a_sb[1, "v"][:, kt, :],
                            start=(kt == 0), stop=(kt == 3),
                        )
                    _vc_copy(nc, vC1, jt, ps)
                for dt_ in range(4):
                    ps = ps_proj.tile([P, 512], f32, tag="proj")
                    for kt in range(4):
                        nc.tensor.matmul(
                            ps,
                            a_sb[1, "q"][:, kt, dt_ * P:(dt_ + 1) * P],
                            h1T[:, kt, 0:ROWS],
                            start=(kt == 0), stop=(kt == 3),
                        )
                    nc.vector.tensor_copy(q1z[0:HD, dt_, 0, :], ps[0:HD, :])
                    nc.vector.tensor_copy(q1z[HD:P, dt_, 1, :], ps[HD:P, :])

        # ------- early cross-attn prep: condT, k2T, v2 (independent of x) ----
        if max_phase >= 4:
            condT = act.tile([P, 4, NCTX], bf16, tag="tE2")
            with (
                tc.tile_pool(name="cin", bufs=3) as cin,
                tc.tile_pool(name="ps_ct", bufs=2, space="PSUM") as ps_ct,
            ):
                for it in range(8):
                    c_sb = cin.tile([P, D], f32, tag="ctile")
                    nc.sync.dma_start(c_sb, condb[:][it * P:(it + 1) * P, :])
                    ct = ps_ct.tile([P, 4, P], f32, tag="ct")
                    for b in range(4):
                        nc.tensor.transpose(
                            ct[:, b, :], c_sb[:, b * P:(b + 1) * P], ident_f32
                        )
                    for b in range(4):
                        nc.scalar.copy(
                            condT[:, b, it * P:(it + 1) * P], ct[:, b, :]
                        )
            k2T = act.tile([P, 4, NCTX], bf16, tag="tX")
            vC2 = _vc_tile(nc, act, "tI", 8)
            with tc.tile_pool(name="ps_proj2a", bufs=4, space="PSUM") as ps_proj:
                for dt_ in range(4):
                    for jc in range(2):
                        ps = ps_proj.tile([P, 512], f32, tag="proj")
                        for kt in range(4):
                            nc.tensor.matmul(
                                ps,
                                a_sb[2, "k"][:, kt, dt_ * P:(dt_ + 1) * P],
                                condT[:, kt, jc * 512:(jc + 1) * 512],
                                start=(kt == 0), stop=(kt == 3),
                            )
                        nc.scalar.copy(
                            k2T[:, dt_, jc * 512:(jc + 1) * 512], ps
                        )
                for jt in range(8):
                    ps = ps_proj.tile([P, 512], f32, tag="proj")
                    for kt in range(4):
                        nc.tensor.matmul(
                            ps,
                            condT[:, kt, jt * P:(jt + 1) * P],
                            a_sb[2, "v"][:, kt, :],
                            start=(kt == 0), stop=(kt == 3),
                        )
                    _vc_copy(nc, vC2, jt, ps)

        # deferred adaln2/adaln3 params: weight DMAs queue behind the
        # attention stacks, PE work lands in the proj -> att1 seam
        ab[2] = _emb(2)
        ab[4] = _emb(4)

        # ---------------- phase 3: attention 1 ---------------------------
        if max_phase >= 3:
            x2 = act.tile([P, 4, D], f32, tag="tF")
            _attention(nc, tc, act, q1z, k1T, vC1, 16, a_sb[1, "o"],
                       a_sb[1, "ob"], ones_row, own_x, x2, "att1")
            final = x2

        # ---------------- phase 4: adaln2 + cross-attn -------------------
        if max_phase >= 4:
            h2T = act.tile([P, 4, ROWS], bf16, tag="tH")
            _adaln_to_hT(nc, tc, lambda it: x2[:, it, :], 4, ab[2], h2T,
                         ident_bf16, eps_sb, "n2")

            q2z = act.tile([P, 4, 2, ROWS], bf16, tag="tE2")
            nc.vector.memset(q2z[HD:P, :, 0, :], 0.0)
            nc.vector.memset(q2z[0:HD, :, 1, :], 0.0)
            with tc.tile_pool(name="ps_proj2b", bufs=2, space="PSUM") as ps_proj:
                for dt_ in range(4):
                    ps = ps_proj.tile([P, 512], f32, tag="proj")
                    for kt in range(4):
                        nc.tensor.matmul(
                            ps,
                            a_sb[2, "q"][:, kt, dt_ * P:(dt_ + 1) * P],
                            h2T[:, kt, :],
                            start=(kt == 0), stop=(kt == 3),
                        )
                    nc.vector.tensor_copy(q2z[0:HD, dt_, 0, :], ps[0:HD, :])
                    nc.vector.tensor_copy(q2z[HD:P, dt_, 1, :], ps[HD:P, :])

            x3 = act.tile([P, 4, D], f32, tag="tG")
            _attention(nc, tc, act, q2z, k2T, vC2, 8, a_sb[2, "o"],
                       a_sb[2, "ob"], ones_row, x2, x3, "att2")
            final = x3

        # ---------------- phase 5: adaln3 + GEGLU FFN --------------------
        if max_phase >= 5:
            h3T = act.tile([P, 4, ROWS], bf16, tag="tD")
            _adaln_to_hT(nc, tc, lambda it: x3[:, it, :], 4, ab[4], h3T,
                         ident_bf16, eps_sb, "n4")

            # ff_w1 halves live in the dead h1T / vC1 slots so their DMAs
            # start as soon as phase 2 / attention-1 stop reading those,
            # instead of waiting for the a1 weight stack to die.
            w1a = act.tile([P, 4, 4 * D], bf16, tag="tA")
            nc.gpsimd.dma_start(
                w1a, ff_w1[:][:, 0:4 * D].rearrange("(k p) n -> p k n", p=P))
            w1b = act.tile([P, 4, 4 * D], bf16, tag="tC")
            nc.gpsimd.dma_start(
                w1b, ff_w1[:][:, 4 * D:8 * D].rearrange("(k p) n -> p k n", p=P))
            w2_sb = wpool.tile([P, 16, D], bf16, tag="wbig2")
            nc.gpsimd.dma_start(w2_sb, ff_w2[:].rearrange("(k p) n -> p k n", p=P))
            b1_sb = const.tile([P, 32], f32)
            nc.sync.dma_start(b1_sb, ff_b1[:].rearrange("(k p) -> p k", p=P))
            b2_row = const.tile([1, D], bf16)
            nc.gpsimd.dma_start(b2_row, ff_b2[:].rearrange("(a n) -> a n", a=1))

            ugT = act.tile([P, 16, ROWS], bf16, tag="tB")
            with (
                tc.tile_pool(name="ps_z", bufs=4, space="PSUM") as ps_z,
                tc.tile_pool(name="gact", bufs=3) as gact_pool,
            ):
                for ut in range(16):
                    zu = ps_z.tile([P, ROWS], f32, tag="z")
                    zg = ps_z.tile([P, ROWS], f32, tag="z")
                    for kt in range(4):
                        nc.tensor.matmul(
                            zu, w1a[:, kt, ut * P:(ut + 1) * P],
                            h3T[:, kt, :], start=(kt == 0), stop=(kt == 3),
                        )
                    for kt in range(4):
                        nc.tensor.matmul(
                            zg, w1b[:, kt, ut * P:(ut + 1) * P],
                            h3T[:, kt, :], start=(kt == 0), stop=(kt == 3),
                        )
                    gact = gact_pool.tile([P, ROWS], bf16, tag="gact")
                    nc.scalar.activation(
                        gact, zg, AF.Gelu, bias=b1_sb[:, 16 + ut:17 + ut], scale=1.0
                    )
                    nc.vector.scalar_tensor_tensor(
                        ugT[:, ut, :], zu, b1_sb[:, ut:ut + 1], gact,
                        op0=OP.add, op1=OP.mult,
                    )

            out_sb = act.tile([P, 4, D], f32, tag="tC")
            with tc.tile_pool(name="ps_y", bufs=2, space="PSUM") as ps_y:
                for it in range(4):
                    ps = ps_y.tile([P, D], f32, tag="y")
                    for kt in range(16):
                        nc.tensor.matmul(
                            ps, ugT[:, kt, it * P:(it + 1) * P],
                            w2_sb[:, kt, :],
                            start=(kt == 0), stop=False,
                        )
                    nc.tensor.matmul(
                        ps, ones_row[0:1, 0:P], b2_row, start=False, stop=True,
                    )
                    nc.vector.tensor_tensor(
                        out_sb[:, it, :], ps, x3[:, it, :], op=OP.add
                    )
            final = out_sb

        for it_ in range(4):
            nc.sync.dma_start(out[:][it_ * P:(it_ + 1) * P, :], final[:, it_, :])

    nc.compile()
    return nc


def _shard_inputs(inputs):
    """Build the 8 per-core input maps."""
    x = np.ascontiguousarray(inputs["x"], dtype=np.float32)
    t = np.ascontiguousarray(inputs["t"], dtype=np.float32)
    cond = np.ascontiguousarray(inputs["cond"], dtype=np.float32)
    shared = {}
    for k in ("n1_w", "n1_b", "n2_w", "n2_b", "n4_w", "n4_b",
              "a1_q", "a1_k", "a1_v", "a1_o", "a1_ob",
              "a2_q", "a2_k", "a2_v", "a2_o", "a2_ob",
              "ff_w1", "ff_b1", "ff_w2", "ff_b2"):
        shared[k] = np.ascontiguousarray(inputs[k], dtype=np.float32)
    in_maps = []
    for c in range(NCORES):
        b = c // 4
        r0 = (c % 4) * ROWS
        m = dict(shared)
        m["xb"] = np.ascontiguousarray(np.roll(x[b], -r0, axis=0))
        m["condb"] = np.ascontiguousarray(cond[b])
        m["t"] = np.ascontiguousarray(t[b, 0])
        in_maps.append(m)
    return in_maps


def kernel(**inputs) -> np.ndarray:
    if "nc" not in _CACHED:
        _CACHED["nc"] = build()
    nc = _CACHED["nc"]
    in_maps = _shard_inputs(inputs)
    res = run_bass_kernel_spmd(nc, in_maps, core_ids=list(range(NCORES)))
    outs = [res.results[c]["out"] for c in range(NCORES)]
    full = np.concatenate(outs, axis=0).reshape(B, N, D)
    return full.astype(np.float32)



# revision 29
# speedup vs baseline: 1.0193x; 1.0193x over previous
"""BasicTransformerBlock on 8 TRN2 NeuronCores.

Sharding: sequence-parallel, zero collectives. The [B=2, N=2048, D=512]
residual stream is split into 8 row-blocks of 512 (4 cores per batch
element). Every core recomputes the cheap batch-wide work it needs
(adaln1 + K/V projections over its batch's 2048 rows, cond K/V), and does
attention / FFN only for its own 512 query rows.

Per-core inputs are pre-rotated with np.roll so that "own" rows are always
rows 0:512 -- the SPMD program is identical on all cores, only data differs.
Attention is permutation-invariant over keys, so rolled K/V is fine.

Layouts (SBUF tiles are [128 partitions, ...]):
  hT  = normed activations, transposed: [128 p=d%128, 4 dtile, rows] bf16
  kT  = [128 p=dout%128, 4 dtile, rows] bf16   (head pair 2t,2t+1 stacked
        in partitions 0:64 / 64:128 of dtile t)
  qz  = [128, 4 ht, 2 slot, rows] bf16: slot s holds head 2ht+s in its own
        64-partition half, the other half ZERO.
  vC  = [128 p=row%128, jt, 4 ht, 193] bf16 overlapped pair tile:
        [v_even(64) | 1 | v_odd(64) | 1 | v_even[0:63]]; head 2ht+s uses
        columns 65s:65s+128. Column 64/129 of each window is 1.0, which
        makes the attention-weight row-sum (softmax denominator) fall out
        of the same matmul that computes attn@v.

Every attention matmul is a full 128x128-array op (scores: K=128 via the
stacked head pair against a zero-padded q half; attn@v: M=128 via the
overlapped v windows). The PE HAM activity monitor only counts full-array
matmuls as "busy": half-array ops (K=64 or M=65) leave the clock gate at
K=4/8 (1.2 GHz effective) for the whole attention phase, which is where
the previous version lost ~120us.

Scores are computed transposed, sT[j, i], so exp() runs on ScalarE straight
out of PSUM and softmax normalization is applied per head on the tiny
attn@v result. Matmul operands are bf16 (weights are cast during the DMA
load by SWDGE); all accumulation/psum/residual math stays fp32.
"""

import contextlib

import numpy as np

import concourse.bass as bass
import concourse.mybir as mybir
import concourse.tile as tile
from concourse import bacc
from concourse.bass_utils import run_bass_kernel_spmd
from concourse.masks import make_identity

dt = mybir.dt
AF = mybir.ActivationFunctionType
OP = mybir.AluOpType

B, N, D = 2, 2048, 512
NCTX = 1024          # cond length
H = 8                # heads
HD = D // H          # 64
EPS = 1e-5
P = 128              # partitions
NCORES = 8
ROWS = 512           # own rows per core
NB = N               # batch rows per core (2048)
SCALE = HD ** -0.5   # 0.125

f32 = dt.float32
bf16 = dt.bfloat16

_CACHED = {}


def _adaln_stats(nc, stat_pool, src_tiles, n_tiles, eps_sb, chunk=4):
    """bn_stats/aggr + rstd/nmr for n_tiles row-tiles. Returns (rstd_all, nmr_all)."""
    mv_all = stat_pool.tile([P, n_tiles, 2], f32)
    rstd_all = stat_pool.tile([P, n_tiles], f32)
    nmr_all = stat_pool.tile([P, n_tiles], f32)
    for c0 in range(0, n_tiles, chunk):
        for it in range(c0, c0 + chunk):
            stats = stat_pool.tile([P, 6], f32, tag="stats")
            nc.vector.bn_stats(stats, src_tiles(it))
            nc.vector.bn_aggr(mv_all[:, it, :], stats)
        cs = slice(c0, c0 + chunk)
        nc.scalar.activation(rstd_all[:, cs], mv_all[:, cs, 1], AF.Sqrt,
                             bias=eps_sb, scale=1.0)
        nc.vector.reciprocal(rstd_all[:, cs], rstd_all[:, cs])
        nc.vector.scalar_tensor_tensor(
            nmr_all[:, cs], mv_all[:, cs, 0], -1.0, rstd_all[:, cs],
            op0=OP.mult, op1=OP.mult,
        )
    return rstd_all, nmr_all


def _adaln_apply(nc, tc, src_tiles, n_tiles, ab, rstd_all, nmr_all, hT,
                 ident_bf16, name):
    """xn = (x-mean)*rstd -> PE transpose -> fused (1+scale)/shift copy."""
    with contextlib.ExitStack() as actx:
        xn_pool = actx.enter_context(tc.tile_pool(name=f"{name}_xn", bufs=3))
        pst_pool = actx.enter_context(
            tc.tile_pool(name=f"{name}_pst", bufs=2, space="PSUM")
        )
        for it in range(n_tiles):
            xn = xn_pool.tile([P, 512], bf16, tag="xn")
            nc.scalar.activation(xn, src_tiles(it), AF.Identity,
                                 bias=nmr_all[:, it:it + 1],
                                 scale=rstd_all[:, it:it + 1])
            xnt = pst_pool.tile([P, 4, P], bf16, tag="xnt")
            for b in range(4):
                nc.tensor.transpose(
                    xnt[:, b, :], xn[:, b * P:(b + 1) * P], ident_bf16
                )
            for b in range(4):
                nc.vector.tensor_scalar(
                    hT[:, b, it * P:(it + 1) * P], xnt[:, b, :],
                    ab[:, b:b + 1], ab[:, 4 + b:5 + b],
                    op0=OP.mult, op1=OP.add,
                )


def _adaln_to_hT(nc, tc, src_tiles, n_tiles, ab, hT, ident_bf16, eps_sb, name):
    with contextlib.ExitStack() as actx:
        stat_pool = actx.enter_context(tc.tile_pool(name=f"{name}_stat", bufs=4))
        rstd_all, nmr_all = _adaln_stats(nc, stat_pool, src_tiles, n_tiles, eps_sb)
        _adaln_apply(nc, tc, src_tiles, n_tiles, ab, rstd_all, nmr_all, hT,
                     ident_bf16, name)


VW = HD + 1       # 65: per-head v window [v_h(64) | 1]
VROW = VW * H     # 520: all 8 head windows of one key tile, contiguous


def _vc_tile(nc, act, pool_tag, njt):
    """Flat v tile: njt contiguous 520-col key-tile rows + 63 tail filler.
    Head h's av weight window is cols [jt*520+65h : +128] = [v_h | 1 |
    v_{h+1}[0:63]] -- M=128 without any extra copies (cols past 65 of a
    window produce garbage psum rows that are never read)."""
    vC = act.tile([P, njt * VROW + HD - 1], bf16, tag=pool_tag)
    nc.vector.memset(
        vC[:, 0:njt * VROW].rearrange("p (j h w) -> p j h w", j=njt, h=H)[:, :, :, HD:VW],
        1.0,
    )
    nc.vector.memset(vC[:, njt * VROW:], 0.0)
    return vC


def _vc_copy(nc, vC, jt, ps):
    dst = vC[:, jt * VROW:(jt + 1) * VROW].rearrange("p (h w) -> p h w", h=H)
    nc.vector.tensor_copy(dst[:, :, 0:HD], ps.rearrange("p (h d) -> p h d", h=H))


def _flush_av(nc, vC, njt, pend, avps, dn_pool, dnms):
    ht, et, jt = pend
    for s in range(2):
        h = 2 * ht + s
        nc.tensor.matmul(
            avps[h], vC[:, jt * VROW + VW * h:jt * VROW + VW * h + P], et[:, s, :],
            start=(jt == 0), stop=(jt == njt - 1),
        )
    if jt == njt - 1:
        # pair finished: pull the denominator rows out so psum banks free
        for s in range(2):
            h = 2 * ht + s
            dnm_h = dn_pool.tile([1, 512], bf16, tag="dnm")
            nc.vector.tensor_copy(dnm_h, avps[h][HD:HD + 1, :])
            dnms[h] = dnm_h


def _attention(nc, tc, act, qz, kT, vC, njt, wo, ob_row, ones_row,
               x_res, x_out, name):
    """Transposed-score attention for 8 heads over own 512 rows.

    qz: [128, 4 ht, 2, rows]; kT: [128, 4, keys]; vC: [128, njt, 4, 193].
    All attention matmuls are full 128x128-array (keeps the HAM clock
    gate open). Writes x_out = attn_out @ wo + ob + x_res.
    """
    av_all = act.tile([P, 4, ROWS], bf16, tag="tH")
    with (
        tc.tile_pool(name=f"{name}_ps_s", bufs=2, space="PSUM") as ps_s,
        tc.tile_pool(name=f"{name}_ps_av", bufs=4, space="PSUM") as ps_av,
        tc.tile_pool(name=f"{name}_et", bufs=3) as et_pool,
        tc.tile_pool(name=f"{name}_dn", bufs=4) as dn_pool,
    ):
        for grp in range(2):
            heads = range(grp * 4, grp * 4 + 4)
            avps = {}
            dnms = {}
            # software-pipelined: scores/exp for step n+1 issue before the
            # av matmuls of step n, so the in-order PE stream never waits on
            # ScalarE's exp latency. The skew also crosses pair boundaries.
            pend = None   # (ht, et, jt)
            for ht in (grp * 2, grp * 2 + 1):
                for s in range(2):
                    avp = ps_av.tile([P, ROWS], f32, tag="av")
                    avps[2 * ht + s] = avp
                for jt in range(njt):
                    sps = ps_s.tile([P, 2, ROWS], f32, tag="s")
                    for s in range(2):
                        nc.tensor.matmul(
                            sps[:, s, :],
                            kT[:, ht, jt * P:(jt + 1) * P],
                            qz[:, ht, s, :],
                            start=True, stop=True,
                        )
                    et = et_pool.tile([P, 2, ROWS], bf16, tag="et")
                    nc.scalar.activation(et, sps, AF.Exp, scale=SCALE)
                    if pend is not None:
                        _flush_av(nc, vC, njt, pend, avps, dn_pool, dnms)
                    pend = (ht, et, jt)
                # flush at pair end is deferred; pend carries over
            if pend is not None:
                _flush_av(nc, vC, njt, pend, avps, dn_pool, dnms)
            # broadcast denom rows across 64 partitions via K=1 matmuls,
            # then one full-width reciprocal per head-pair (psum -> sbuf)
            rbs = {}
            for pair in range(2):
                rb = ps_s.tile([P, 2, ROWS], f32, tag="s")
                for sub in range(2):
                    hh = grp * 4 + pair * 2 + sub
                    nc.tensor.matmul(
                        rb[sub * HD:(sub + 1) * HD, 0, :],
                        ones_row[0:1, 0:HD],
                        dnms[hh][0:1, :],
                        start=True, stop=True,
                    )
                rb_sb = et_pool.tile([P, ROWS], bf16, tag="rb")
                with nc.allow_low_precision(reason="bf16 softmax recip"):
                    nc.vector.reciprocal(rb_sb, rb[:, 0, :])
                rbs[pair] = rb_sb
            for h in heads:
                po = 64 * (h % 2)
                nc.vector.scalar_tensor_tensor(
                    av_all[po:po + HD, h // 2, :],
                    avps[h][0:HD, :], 1.0, rbs[(h % 4) // 2][po:po + HD, :],
                    op0=OP.mult, op1=OP.mult,
                )
    # out-projection + bias + residual
    with tc.tile_pool(name=f"{name}_ps_o", bufs=2, space="PSUM") as ps_o:
        for it in range(4):
            ps = ps_o.tile([P, D], f32, tag="o")
            for dt_ in range(4):
                nc.tensor.matmul(
                    ps, av_all[:, dt_, it * P:(it + 1) * P], wo[:, dt_, :],
                    start=(dt_ == 0), stop=False,
                )
            nc.tensor.matmul(
                ps, ones_row[0:1, 0:P], ob_row, start=False, stop=True,
            )
            nc.vector.tensor_tensor(x_out[:, it, :], ps, x_res[:, it, :], op=OP.add)


def build(max_phase=5):
    nc = bacc.Bacc(None, target_bir_lowering=False)

    # ---------------- I/O ----------------
    xb = nc.dram_tensor("xb", [NB, D], f32, kind="ExternalInput")
    condb = nc.dram_tensor("condb", [NCTX, D], f32, kind="ExternalInput")
    t_in = nc.dram_tensor("t", [D], f32, kind="ExternalInput")
    nw = {}
    nb_ = {}
    for l in (1, 2, 4):
        nw[l] = nc.dram_tensor(f"n{l}_w", [D, 2 * D], f32, kind="ExternalInput")
        nb_[l] = nc.dram_tensor(f"n{l}_b", [2 * D], f32, kind="ExternalInput")
    aw = {}
    for a in (1, 2):
        for w in "qkvo":
            aw[a, w] = nc.dram_tensor(f"a{a}_{w}", [D, D], f32, kind="ExternalInput")
        aw[a, "ob"] = nc.dram_tensor(f"a{a}_ob", [D], f32, kind="ExternalInput")
    ff_w1 = nc.dram_tensor("ff_w1", [D, 8 * D], f32, kind="ExternalInput")
    ff_b1 = nc.dram_tensor("ff_b1", [8 * D], f32, kind="ExternalInput")
    ff_w2 = nc.dram_tensor("ff_w2", [4 * D, D], f32, kind="ExternalInput")
    ff_b2 = nc.dram_tensor("ff_b2", [D], f32, kind="ExternalInput")
    out = nc.dram_tensor("out", [ROWS, D], f32, kind="ExternalOutput")

    with tile.TileContext(nc) as tc, contextlib.ExitStack() as ctx:
        const = ctx.enter_context(tc.tile_pool(name="const", bufs=1))
        wpool = ctx.enter_context(tc.tile_pool(name="wpool", bufs=1))
        act = ctx.enter_context(tc.tile_pool(name="act", bufs=1))

        ident_bf16 = const.tile([P, P], bf16)
        make_identity(nc, ident_bf16)
        ident_f32 = const.tile([P, P], f32)
        make_identity(nc, ident_f32)
        ones_row = const.tile([1, P], bf16)
        nc.vector.memset(ones_row, 1.0)
        eps_sb = const.tile([P, 1], f32)
        nc.vector.memset(eps_sb, EPS)

        # PE warmup: ~50 dependency-free matmuls fill the otherwise idle
        # startup window and lift the HAM clock gate to 2.4 GHz early
        with tc.tile_pool(name="warm", bufs=1, space="PSUM") as warm_pool:
            wps = warm_pool.tile([P, P], f32)
            for _ in range(50):
                nc.tensor.matmul(wps, ident_bf16, ident_bf16,
                                 start=True, stop=True)

        # t as column tiles [128, 4] bf16 for emb matmul lhsT
        tT = const.tile([P, 4], bf16)
        nc.gpsimd.dma_start(tT, t_in[:].rearrange("(k p) -> p k", p=P))

        h1T = act.tile([P, 4, NB], bf16, tag="tA")
        own_x = act.tile([P, 4, D], f32, tag="tE")
        # non-own rows only feed adaln1 -> K/V; bf16 is plenty and halves
        # their SBUF footprint (the DMA casts f32->bf16 in flight)
        xrest = act.tile([P, 12, D], bf16, tag="tX")
        x_tiles = {}
        for it in range(16):
            if it < 4:
                dst = own_x[:, it, :]
                nc.sync.dma_start(dst, xb[:][it * P:(it + 1) * P, :])
            else:
                dst = xrest[:, it - 4, :]
                nc.gpsimd.dma_start(dst, xb[:][it * P:(it + 1) * P, :])
            x_tiles[it] = dst

        # adaln1 stats issue first: independent of norm weights, keeps DVE
        # busy while the emb chain waits on its weight DMAs
        n1_stat = ctx.enter_context(tc.tile_pool(name="n1_stat", bufs=4))
        if max_phase >= 1:
            rstd1, nmr1 = _adaln_stats(nc, n1_stat, lambda it: x_tiles[it],
                                       16, eps_sb)

        # ---------------- norm scale/shift params ----------------
        # emb = t @ nw + nb  -> [1, 1024] -> [128, 8] columns. Only layer 1
        # is needed early; layers 2/4 are deferred past the phase-2 issue so
        # their 2.1MB weight DMAs queue behind the attention weights.
        def _emb(l):
            with (
                tc.tile_pool(name=f"nwp{l}", bufs=1) as nwp,
                tc.tile_pool(name=f"embp{l}", bufs=1) as embp,
                tc.tile_pool(name=f"ps_emb{l}", bufs=2, space="PSUM") as ps_emb,
            ):
                nw_sb = nwp.tile([P, 4, 2 * D], bf16, tag="nw")
                nc.gpsimd.dma_start(
                    nw_sb, nw[l][:].rearrange("(k p) n -> p k n", p=P)
                )
                nb_row = embp.tile([1, 2 * D], f32, tag="nbrow")
                nc.sync.dma_start(nb_row, nb_[l][:].rearrange("(a n) -> a n", a=1))
                emb_ps = ps_emb.tile([1, 2 * D], f32, tag="embps")
                for half in range(2):
                    for kt in range(4):
                        nc.tensor.matmul(
                            emb_ps[:, half * D:(half + 1) * D],
                            tT[:, kt:kt + 1],
                            nw_sb[:, kt, half * D:(half + 1) * D],
                            start=(kt == 0), stop=(kt == 3),
                        )
                emb_row = embp.tile([1, 2 * D], f32, tag="embrow")
                nc.vector.tensor_tensor(emb_row, emb_ps, nb_row, op=OP.add)
                # row -> per-partition columns via tiny PE transposes;
                # scale columns (0:4) get the +1 fused into the psum copy
                ab_l = const.tile([P, 8], f32, tag=f"ab{l}")
                for col in range(8):
                    tp = ps_emb.tile([P, 1], f32, tag="embT")
                    nc.tensor.transpose(
                        tp, emb_row[0:1, col * P:(col + 1) * P],
                        ident_f32[0:1, 0:1]
                    )
                    nc.vector.tensor_scalar(
                        ab_l[:, col:col + 1], tp,
                        1.0 if col < 4 else 0.0, None, op0=OP.add,
                    )
            return ab_l

        ab = {1: _emb(1)}



        # ---------------- attention weights (bf16 via DMA cast) ----------
        # a1 stack shares addresses with ff_w1, a2 stack with ff_w2
        # (sequential lifetimes; Tile inserts the WAR deps).
        a_sb = {}
        for a, wtag in ((1, "wbig1"), (2, "wbig2")):
            stack = wpool.tile([P, 4, 4, D], bf16, tag=wtag)
            for wi, w in enumerate("qkvo"):
                nc.gpsimd.dma_start(
                    stack[:, :, wi, :],
                    aw[a, w][:].rearrange("(k p) n -> p k n", p=P),
                )
                a_sb[a, w] = stack[:, :, wi, :]
            ob = wpool.tile([1, D], bf16, tag=f"a{a}ob")
            nc.gpsimd.dma_start(ob, aw[a, "ob"][:].rearrange("(a n) -> a n", a=1))
            a_sb[a, "ob"] = ob


        # ---------------- phase 1: adaln1 apply -> h1T -------------------
        if max_phase < 1:
            final = own_x
        else:
            _adaln_apply(nc, tc, lambda it: x_tiles[it], 16, ab[1], rstd1,
                         nmr1, h1T, ident_bf16, "n1")
            final = own_x

        # ---------------- phase 2: projections k1T, vC1, q1z -------------
        if max_phase >= 2:
            k1T = act.tile([P, 4, NB], bf16, tag="tB")
            vC1 = _vc_tile(nc, act, "tC", 16)
            q1z = act.tile([P, 4, 2, ROWS], bf16, tag="tD")
            nc.vector.memset(q1z[HD:P, :, 0, :], 0.0)
            nc.vector.memset(q1z[0:HD, :, 1, :], 0.0)
            with tc.tile_pool(name="ps_proj1", bufs=4, space="PSUM") as ps_proj:
                for dt_ in range(4):
                    for jc in range(4):
                        ps = ps_proj.tile([P, 512], f32, tag="proj")
                        for kt in range(4):
                            nc.tensor.matmul(
                                ps,
                                a_sb[1, "k"][:, kt, dt_ * P:(dt_ + 1) * P],
                                h1T[:, kt, jc * 512:(jc + 1) * 512],
                                start=(kt == 0), stop=(kt == 3),
                            )
                        nc.scalar.copy(
                            k1T[:, dt_, jc * 512:(jc + 1) * 512], ps
                        )
                for jt in range(16):
                    ps = ps_proj.tile([P, 512], f32, tag="proj")
                    for kt in range(4):
                        nc.tensor.matmul(
                            ps,
                            h1T[:, kt, jt * P:(jt + 1) * P],
                            a_sb[1, "v"][:, kt, :],
                            start=(kt == 0), stop=(kt == 3),
                        )
                    _vc_copy(nc, vC1, jt, ps)
                for dt_ in range(4):
                    ps = ps_proj.tile([P, 512], f32, tag="proj")
                    for kt in range(4):
                        nc.tensor.matmul(
                            ps,
                            a_sb[1, "q"][:, kt, dt_ * P:(dt_ + 1) * P],
                            h1T[:, kt, 0:ROWS],
                            start=(kt == 0), stop=(kt == 3),
                        )
                    nc.vector.tensor_copy(q1z[0:HD, dt_, 0, :], ps[0:HD, :])
                    nc.vector.tensor_copy(q1z[HD:P, dt_, 1, :], ps[HD:P, :])

        # ------- early cross-attn prep: condT, k2T, v2 (independent of x) ----
        if max_phase >= 4:
            condT = act.tile([P, 4, NCTX], bf16, tag="tE2")
            with (
                tc.tile_pool(name="cin", bufs=3) as cin,
                tc.tile_pool(name="ps_ct", bufs=2, space="PSUM") as ps_ct,
            ):
                for it in range(8):
                    c_sb = cin.tile([P, D], f32, tag="ctile")
                    nc.sync.dma_start(c_sb, condb[:][it * P:(it + 1) * P, :])
                    ct = ps_ct.tile([P, 4, P], f32, tag="ct")
                    for b in range(4):
                        nc.tensor.transpose(
                            ct[:, b, :], c_sb[:, b * P:(b + 1) * P], ident_f32
                        )
                    for b in range(4):
                        nc.scalar.copy(
                            condT[:, b, it * P:(it + 1) * P], ct[:, b, :]
                        )
            k2T = act.tile([P, 4, NCTX], bf16, tag="tX")
            vC2 = _vc_tile(nc, act, "tI", 8)
            with tc.tile_pool(name="ps_proj2a", bufs=4, space="PSUM") as ps_proj:
                for dt_ in range(4):
                    for jc in range(2):
                        ps = ps_proj.tile([P, 512], f32, tag="proj")
                        for kt in range(4):
                            nc.tensor.matmul(
                                ps,
                                a_sb[2, "k"][:, kt, dt_ * P:(dt_ + 1) * P],
                                condT[:, kt, jc * 512:(jc + 1) * 512],
                                start=(kt == 0), stop=(kt == 3),
                            )
                        nc.scalar.copy(
                            k2T[:, dt_, jc * 512:(jc + 1) * 512], ps
                        )
                for jt in range(8):
                    ps = ps_proj.tile([P, 512], f32, tag="proj")
                    for kt in range(4):
                        nc.tensor.matmul(
                            ps,
                            condT[:, kt, jt * P:(jt + 1) * P],
                            a_sb[2, "v"][:, kt, :],
                            start=(kt == 0), stop=(kt == 3),
                        )
                    _vc_copy(nc, vC2, jt, ps)

        # deferred adaln2/adaln3 params: weight DMAs queue behind the
        # attention stacks, PE work lands in the proj -> att1 seam
        ab[2] = _emb(2)
        ab[4] = _emb(4)

        # ---------------- phase 3: attention 1 ---------------------------
        if max_phase >= 3:
            x2 = act.tile([P, 4, D], f32, tag="tF")
            _attention(nc, tc, act, q1z, k1T, vC1, 16, a_sb[1, "o"],
                       a_sb[1, "ob"], ones_row, own_x, x2, "att1")
            final = x2

        # ---------------- phase 4: adaln2 + cross-attn -------------------
        if max_phase >= 4:
            h2T = act.tile([P, 4, ROWS], bf16, tag="tH")
            _adaln_to_hT(nc, tc, lambda it: x2[:, it, :], 4, ab[2], h2T,
                         ident_bf16, eps_sb, "n2")

            q2z = act.tile([P, 4, 2, ROWS], bf16, tag="tE2")
            nc.vector.memset(q2z[HD:P, :, 0, :], 0.0)
            nc.vector.memset(q2z[0:HD, :, 1, :], 0.0)
            with tc.tile_pool(name="ps_proj2b", bufs=2, space="PSUM") as ps_proj:
                for dt_ in range(4):
                    ps = ps_proj.tile([P, 512], f32, tag="proj")
                    for kt in range(4):
                        nc.tensor.matmul(
                            ps,
                            a_sb[2, "q"][:, kt, dt_ * P:(dt_ + 1) * P],
                            h2T[:, kt, :],
                            start=(kt == 0), stop=(kt == 3),
                        )
                    nc.vector.tensor_copy(q2z[0:HD, dt_, 0, :], ps[0:HD, :])
                    nc.vector.tensor_copy(q2z[HD:P, dt_, 1, :], ps[HD:P, :])

            x3 = act.tile([P, 4, D], f32, tag="tG")
            _attention(nc, tc, act, q2z, k2T, vC2, 8, a_sb[2, "o"],
                       a_sb[2, "ob"], ones_row, x2, x3, "att2")
            final = x3

        # ---------------- phase 5: adaln3 + GEGLU FFN --------------------
        if max_phase >= 5:
            h3T = act.tile([P, 4, ROWS], bf16, tag="tD")
            _adaln_to_hT(nc, tc, lambda it: x3[:, it, :], 4, ab[4], h3T,
                         ident_bf16, eps_sb, "n4")

            # ff_w1 halves live in the dead h1T / vC1 slots so their DMAs
            # start as soon as phase 2 / attention-1 stop reading those,
            # instead of waiting for the a1 weight stack to die.
            w1a = act.tile([P, 4, 4 * D], bf16, tag="tA")
            nc.gpsimd.dma_start(
                w1a, ff_w1[:][:, 0:4 * D].rearrange("(k p) n -> p k n", p=P))
            w1b = act.tile([P, 4, 4 * D], bf16, tag="tC")
            nc.gpsimd.dma_start(
                w1b, ff_w1[:][:, 4 * D:8 * D].rearrange("(k p) n -> p k n", p=P))
            w2_sb = wpool.tile([P, 16, D], bf16, tag="wbig2")
            nc.gpsimd.dma_start(w2_sb, ff_w2[:].rearrange("(k p) n -> p k n", p=P))
            b1_sb = const.tile([P, 32], f32)
            nc.sync.dma_start(b1_sb, ff_b1[:].rearrange("(k p) -> p k", p=P))
            b2_row = const.tile([1, D], bf16)
            nc.gpsimd.dma_start(b2_row, ff_b2[:].rearrange("(a n) -> a n", a=1))

            ugT = act.tile([P, 16, ROWS], bf16, tag="tB")
            with (
                tc.tile_pool(name="ps_z", bufs=4, space="PSUM") as ps_z,
                tc.tile_pool(name="gact", bufs=3) as gact_pool,
            ):
                for ut in range(16):
                    zu = ps_z.tile([P, ROWS], f32, tag="z")
                    zg = ps_z.tile([P, ROWS], f32, tag="z")
                    for kt in range(4):
                        nc.tensor.matmul(
                            zu, w1a[:, kt, ut * P:(ut + 1) * P],
                            h3T[:, kt, :], start=(kt == 0), stop=(kt == 3),
                        )
                    for kt in range(4):
                        nc.tensor.matmul(
                            zg, w1b[:, kt, ut * P:(ut + 1) * P],
                            h3T[:, kt, :], start=(kt == 0), stop=(kt == 3),
                        )
                    gact = gact_pool.tile([P, ROWS], bf16, tag="gact")
                    nc.scalar.activation(
                        gact, zg, AF.Gelu, bias=b1_sb[:, 16 + ut:17 + ut], scale=1.0
                    )
                    nc.vector.scalar_tensor_tensor(
                        ugT[:, ut, :], zu, b1_sb[:, ut:ut + 1], gact,
                        op0=OP.add, op1=OP.mult,
                    )

            out_sb = act.tile([P, 4, D], f32, tag="tC")
            with tc.tile_pool(name="ps_y", bufs=2, space="PSUM") as ps_y:
                for it in range(4):
                    ps = ps_y.tile([P, D], f32, tag="y")
                    for kt in range(16):
                        nc.tensor.matmul(
                            ps, ugT[:, kt, it * P:(it + 1) * P],
                            w2_sb[:, kt, :],
                            start=(kt == 0), stop=False,
                        )
                    nc.tensor.matmul(
                        ps, ones_row[0:1, 0:P], b2_row, start=False, stop=True,
                    )
                    nc.vector.tensor_tensor(
                        out_sb[:, it, :], ps, x3[:, it, :], op=OP.add
                    )
            final = out_sb

        for it_ in range(4):
            nc.sync.dma_start(out[:][it_ * P:(it_ + 1) * P, :], final[:, it_, :])

    nc.compile()
    return nc


def _shard_inputs(inputs):
    """Build the 8 per-core input maps."""
    x = np.ascontiguousarray(inputs["x"], dtype=np.float32)
    t = np.ascontiguousarray(inputs["t"], dtype=np.float32)
    cond = np.ascontiguousarray(inputs["cond"], dtype=np.float32)
    shared = {}
    for k in ("n1_w", "n1_b", "n2_w", "n2_b", "n4_w", "n4_b",
              "a1_q", "a1_k", "a1_v", "a1_o", "a1_ob",
              "a2_q", "a2_k", "a2_v", "a2_o", "a2_ob",
              "ff_w1", "ff_b1", "ff_w2", "ff_b2"):
        shared[k] = np.ascontiguousarray(inputs[k], dtype=np.float32)
    in_maps = []
    for c in range(NCORES):
        b = c // 4
        r0 = (c % 4) * ROWS
        m = dict(shared)
        m["xb"] = np.ascontiguousarray(np.roll(x[b], -r0, axis=0))
        m["condb"] = np.ascontiguousarray(cond[b])
        m["t"] = np.ascontiguousarray(t[b, 0])
        in_maps.append(m)
    return in_maps


def kernel(**inputs) -> np.ndarray:
    if "nc" not in _CACHED:
        _CACHED["nc"] = build()
    nc = _CACHED["nc"]
    in_maps = _shard_inputs(inputs)
    res = run_bass_kernel_spmd(nc, in_maps, core_ids=list(range(NCORES)))
    outs = [res.results[c]["out"] for c in range(NCORES)]
    full = np.concatenate(outs, axis=0).reshape(B, N, D)
    return full.astype(np.float32)



# revision 32
# speedup vs baseline: 1.0319x; 1.0124x over previous
"""BasicTransformerBlock on 8 TRN2 NeuronCores.

Sharding: sequence-parallel, zero collectives. The [B=2, N=2048, D=512]
residual stream is split into 8 row-blocks of 512 (4 cores per batch
element). Every core recomputes the cheap batch-wide work it needs
(adaln1 + K/V projections over its batch's 2048 rows, cond K/V), and does
attention / FFN only for its own 512 query rows.

Per-core inputs are pre-rotated with np.roll so that "own" rows are always
rows 0:512 -- the SPMD program is identical on all cores, only data differs.
Attention is permutation-invariant over keys, so rolled K/V is fine.

Layouts (SBUF tiles are [128 partitions, ...]):
  hT  = normed activations, transposed: [128 p=d%128, 4 dtile, rows] bf16
  kT  = [128 p=dout%128, 4 dtile, rows] bf16   (head pair 2t,2t+1 stacked
        in partitions 0:64 / 64:128 of dtile t)
  qz  = [128, 4 ht, 2 slot, rows] bf16: slot s holds head 2ht+s in its own
        64-partition half, the other half ZERO.
  vC  = [128 p=row%128, jt, 4 ht, 193] bf16 overlapped pair tile:
        [v_even(64) | 1 | v_odd(64) | 1 | v_even[0:63]]; head 2ht+s uses
        columns 65s:65s+128. Column 64/129 of each window is 1.0, which
        makes the attention-weight row-sum (softmax denominator) fall out
        of the same matmul that computes attn@v.

Every attention matmul is a full 128x128-array op (scores: K=128 via the
stacked head pair against a zero-padded q half; attn@v: M=128 via the
overlapped v windows). The PE HAM activity monitor only counts full-array
matmuls as "busy": half-array ops (K=64 or M=65) leave the clock gate at
K=4/8 (1.2 GHz effective) for the whole attention phase, which is where
the previous version lost ~120us.

Scores are computed transposed, sT[j, i], so exp() runs on ScalarE straight
out of PSUM and softmax normalization is applied per head on the tiny
attn@v result. Matmul operands are bf16 (weights are cast during the DMA
load by SWDGE); all accumulation/psum/residual math stays fp32.
"""

import contextlib

import numpy as np

import concourse.bass as bass
import concourse.mybir as mybir
import concourse.tile as tile
from concourse import bacc
from concourse.bass_utils import run_bass_kernel_spmd
from concourse.masks import make_identity

dt = mybir.dt
AF = mybir.ActivationFunctionType
OP = mybir.AluOpType

B, N, D = 2, 2048, 512
NCTX = 1024          # cond length
H = 8                # heads
HD = D // H          # 64
EPS = 1e-5
P = 128              # partitions
NCORES = 8
ROWS = 512           # own rows per core
NB = N               # batch rows per core (2048)
SCALE = HD ** -0.5   # 0.125

f32 = dt.float32
bf16 = dt.bfloat16

_CACHED = {}


def _adaln_stats(nc, stat_pool, src_tiles, n_tiles, eps_sb, chunk=4):
    """bn_stats/aggr + rstd/nmr for n_tiles row-tiles. Returns (rstd_all, nmr_all)."""
    mv_all = stat_pool.tile([P, n_tiles, 2], f32)
    rstd_all = stat_pool.tile([P, n_tiles], f32)
    nmr_all = stat_pool.tile([P, n_tiles], f32)
    for c0 in range(0, n_tiles, chunk):
        for it in range(c0, c0 + chunk):
            stats = stat_pool.tile([P, 6], f32, tag="stats")
            nc.vector.bn_stats(stats, src_tiles(it))
            nc.vector.bn_aggr(mv_all[:, it, :], stats)
        cs = slice(c0, c0 + chunk)
        nc.scalar.activation(rstd_all[:, cs], mv_all[:, cs, 1], AF.Sqrt,
                             bias=eps_sb, scale=1.0)
        nc.vector.reciprocal(rstd_all[:, cs], rstd_all[:, cs])
        nc.vector.scalar_tensor_tensor(
            nmr_all[:, cs], mv_all[:, cs, 0], -1.0, rstd_all[:, cs],
            op0=OP.mult, op1=OP.mult,
        )
    return rstd_all, nmr_all


def _adaln_apply(nc, tc, src_tiles, n_tiles, ab, rstd_all, nmr_all, hT,
                 ident_bf16, name):
    """xn = (x-mean)*rstd -> PE transpose -> fused (1+scale)/shift copy."""
    with contextlib.ExitStack() as actx:
        xn_pool = actx.enter_context(tc.tile_pool(name=f"{name}_xn", bufs=3))
        pst_pool = actx.enter_context(
            tc.tile_pool(name=f"{name}_pst", bufs=2, space="PSUM")
        )
        for it in range(n_tiles):
            xn = xn_pool.tile([P, 512], bf16, tag="xn")
            nc.scalar.activation(xn, src_tiles(it), AF.Identity,
                                 bias=nmr_all[:, it:it + 1],
                                 scale=rstd_all[:, it:it + 1])
            xnt = pst_pool.tile([P, 4, P], bf16, tag="xnt")
            for b in range(4):
                nc.tensor.transpose(
                    xnt[:, b, :], xn[:, b * P:(b + 1) * P], ident_bf16
                )
            for b in range(4):
                nc.vector.tensor_scalar(
                    hT[:, b, it * P:(it + 1) * P], xnt[:, b, :],
                    ab[:, b:b + 1], ab[:, 4 + b:5 + b],
                    op0=OP.mult, op1=OP.add,
                )


def _adaln_to_hT(nc, tc, src_tiles, n_tiles, ab, hT, ident_bf16, eps_sb, name):
    with contextlib.ExitStack() as actx:
        stat_pool = actx.enter_context(tc.tile_pool(name=f"{name}_stat", bufs=4))
        rstd_all, nmr_all = _adaln_stats(nc, stat_pool, src_tiles, n_tiles, eps_sb)
        _adaln_apply(nc, tc, src_tiles, n_tiles, ab, rstd_all, nmr_all, hT,
                     ident_bf16, name)


VW = HD + 1       # 65: per-head v window [v_h(64) | 1]
VROW = VW * H     # 520: all 8 head windows of one key tile, contiguous


def _vc_tile(nc, act, pool_tag, njt):
    """Flat v tile: njt contiguous 520-col key-tile rows + 63 tail filler.
    Head h's av weight window is cols [jt*520+65h : +128] = [v_h | 1 |
    v_{h+1}[0:63]] -- M=128 without any extra copies (cols past 65 of a
    window produce garbage psum rows that are never read)."""
    vC = act.tile([P, njt * VROW + HD - 1], bf16, tag=pool_tag)
    nc.vector.memset(
        vC[:, 0:njt * VROW].rearrange("p (j h w) -> p j h w", j=njt, h=H)[:, :, :, HD:VW],
        1.0,
    )
    nc.vector.memset(vC[:, njt * VROW:], 0.0)
    return vC


def _vc_copy(nc, vC, jt, ps):
    dst = vC[:, jt * VROW:(jt + 1) * VROW].rearrange("p (h w) -> p h w", h=H)
    nc.vector.tensor_copy(dst[:, :, 0:HD], ps.rearrange("p (h d) -> p h d", h=H))


def _flush_av(nc, vC, njt, pend, avps, dn_pool, dnms):
    ht, et, jt = pend
    for s in range(2):
        h = 2 * ht + s
        nc.tensor.matmul(
            avps[h], vC[:, jt * VROW + VW * h:jt * VROW + VW * h + P], et[:, s, :],
            start=(jt == 0), stop=(jt == njt - 1),
        )
    if jt == njt - 1:
        # pair finished: pull the denominator rows out so psum banks free
        for s in range(2):
            h = 2 * ht + s
            dnm_h = dn_pool.tile([1, 512], bf16, tag="dnm")
            nc.vector.tensor_copy(dnm_h, avps[h][HD:HD + 1, :])
            dnms[h] = dnm_h


def _attention(nc, tc, act, qz, kT, vC, njt, wo, ob_row, ones_row,
               x_res, x_out, name):
    """Transposed-score attention for 8 heads over own 512 rows.

    qz: [128, 4 ht, 2, rows]; kT: [128, 4, keys]; vC: [128, njt, 4, 193].
    All attention matmuls are full 128x128-array (keeps the HAM clock
    gate open). Writes x_out = attn_out @ wo + ob + x_res.
    """
    av_all = act.tile([P, 4, ROWS], bf16, tag="tH")
    with (
        tc.tile_pool(name=f"{name}_ps_s", bufs=2, space="PSUM") as ps_s,
        tc.tile_pool(name=f"{name}_ps_av", bufs=4, space="PSUM") as ps_av,
        tc.tile_pool(name=f"{name}_et", bufs=3) as et_pool,
        tc.tile_pool(name=f"{name}_dn", bufs=4) as dn_pool,
    ):
        for grp in range(2):
            heads = range(grp * 4, grp * 4 + 4)
            avps = {}
            dnms = {}
            # software-pipelined: scores/exp for step n+1 issue before the
            # av matmuls of step n, so the in-order PE stream never waits on
            # ScalarE's exp latency. The skew also crosses pair boundaries.
            pend = None   # (ht, et, jt)
            for ht in (grp * 2, grp * 2 + 1):
                for s in range(2):
                    avp = ps_av.tile([P, ROWS], f32, tag="av")
                    avps[2 * ht + s] = avp
                for jt in range(njt):
                    sps = ps_s.tile([P, 2, ROWS], f32, tag="s")
                    for s in range(2):
                        nc.tensor.matmul(
                            sps[:, s, :],
                            kT[:, ht, jt * P:(jt + 1) * P],
                            qz[:, ht, s, :],
                            start=True, stop=True,
                        )
                    et = et_pool.tile([P, 2, ROWS], bf16, tag="et")
                    nc.scalar.activation(et, sps, AF.Exp, scale=SCALE)
                    if pend is not None:
                        _flush_av(nc, vC, njt, pend, avps, dn_pool, dnms)
                    pend = (ht, et, jt)
                # flush at pair end is deferred; pend carries over
            if pend is not None:
                _flush_av(nc, vC, njt, pend, avps, dn_pool, dnms)
            # broadcast denom rows across 64 partitions via K=1 matmuls,
            # then one full-width reciprocal per head-pair (psum -> sbuf)
            rbs = {}
            for pair in range(2):
                rb = ps_s.tile([P, 2, ROWS], f32, tag="s")
                for sub in range(2):
                    hh = grp * 4 + pair * 2 + sub
                    nc.tensor.matmul(
                        rb[sub * HD:(sub + 1) * HD, 0, :],
                        ones_row[0:1, 0:HD],
                        dnms[hh][0:1, :],
                        start=True, stop=True,
                    )
                rb_sb = et_pool.tile([P, ROWS], bf16, tag="rb")
                with nc.allow_low_precision(reason="bf16 softmax recip"):
                    nc.vector.reciprocal(rb_sb, rb[:, 0, :])
                rbs[pair] = rb_sb
            for h in heads:
                po = 64 * (h % 2)
                nc.vector.scalar_tensor_tensor(
                    av_all[po:po + HD, h // 2, :],
                    avps[h][0:HD, :], 1.0, rbs[(h % 4) // 2][po:po + HD, :],
                    op0=OP.mult, op1=OP.mult,
                )
    # out-projection + bias + residual
    with tc.tile_pool(name=f"{name}_ps_o", bufs=2, space="PSUM") as ps_o:
        for it in range(4):
            ps = ps_o.tile([P, D], f32, tag="o")
            for dt_ in range(4):
                nc.tensor.matmul(
                    ps, av_all[:, dt_, it * P:(it + 1) * P], wo[:, dt_, :],
                    start=(dt_ == 0), stop=False,
                )
            nc.tensor.matmul(
                ps, ones_row[0:1, 0:P], ob_row, start=False, stop=True,
            )
            nc.vector.tensor_tensor(x_out[:, it, :], ps, x_res[:, it, :], op=OP.add)


def build(max_phase=5):
    nc = bacc.Bacc(None, target_bir_lowering=False)

    # ---------------- I/O ----------------
    xb = nc.dram_tensor("xb", [NB, D], f32, kind="ExternalInput")
    condb = nc.dram_tensor("condb", [NCTX, D], f32, kind="ExternalInput")
    t_in = nc.dram_tensor("t", [D], f32, kind="ExternalInput")
    nw = {}
    nb_ = {}
    for l in (1, 2, 4):
        nw[l] = nc.dram_tensor(f"n{l}_w", [D, 2 * D], f32, kind="ExternalInput")
        nb_[l] = nc.dram_tensor(f"n{l}_b", [2 * D], f32, kind="ExternalInput")
    aw = {}
    for a in (1, 2):
        for w in "qkvo":
            aw[a, w] = nc.dram_tensor(f"a{a}_{w}", [D, D], f32, kind="ExternalInput")
        aw[a, "ob"] = nc.dram_tensor(f"a{a}_ob", [D], f32, kind="ExternalInput")
    ff_w1 = nc.dram_tensor("ff_w1", [D, 8 * D], f32, kind="ExternalInput")
    ff_b1 = nc.dram_tensor("ff_b1", [8 * D], f32, kind="ExternalInput")
    ff_w2 = nc.dram_tensor("ff_w2", [4 * D, D], f32, kind="ExternalInput")
    ff_b2 = nc.dram_tensor("ff_b2", [D], f32, kind="ExternalInput")
    out = nc.dram_tensor("out", [ROWS, D], f32, kind="ExternalOutput")

    with tile.TileContext(nc) as tc, contextlib.ExitStack() as ctx:
        const = ctx.enter_context(tc.tile_pool(name="const", bufs=1))
        wpool = ctx.enter_context(tc.tile_pool(name="wpool", bufs=1))
        act = ctx.enter_context(tc.tile_pool(name="act", bufs=1))

        ident_bf16 = const.tile([P, P], bf16)
        make_identity(nc, ident_bf16)
        ident_f32 = const.tile([P, P], f32)
        make_identity(nc, ident_f32)
        ones_row = const.tile([1, P], bf16)
        nc.vector.memset(ones_row, 1.0)
        eps_sb = const.tile([P, 1], f32)
        nc.vector.memset(eps_sb, EPS)

        # PE warmup: ~50 dependency-free matmuls fill the otherwise idle
        # startup window and lift the HAM clock gate to 2.4 GHz early
        with tc.tile_pool(name="warm", bufs=1, space="PSUM") as warm_pool:
            wps = warm_pool.tile([P, P], f32)
            for _ in range(50):
                nc.tensor.matmul(wps, ident_bf16, ident_bf16,
                                 start=True, stop=True)

        # t as column tiles [128, 4] bf16 for emb matmul lhsT
        tT = const.tile([P, 4], bf16)
        nc.gpsimd.dma_start(tT, t_in[:].rearrange("(k p) -> p k", p=P))

        h1T = act.tile([P, 4, NB], bf16, tag="tA")
        own_x = act.tile([P, 4, D], f32, tag="tE")
        # non-own rows only feed adaln1 -> K/V; bf16 is plenty and halves
        # their SBUF footprint (the DMA casts f32->bf16 in flight)
        xrest = act.tile([P, 12, D], bf16, tag="tX")
        x_tiles = {}
        for it in range(16):
            if it < 4:
                dst = own_x[:, it, :]
                nc.sync.dma_start(dst, xb[:][it * P:(it + 1) * P, :])
            else:
                dst = xrest[:, it - 4, :]
                nc.gpsimd.dma_start(dst, xb[:][it * P:(it + 1) * P, :])
            x_tiles[it] = dst

        # adaln1 stats issue first: independent of norm weights, keeps DVE
        # busy while the emb chain waits on its weight DMAs
        n1_stat = ctx.enter_context(tc.tile_pool(name="n1_stat", bufs=4))
        if max_phase >= 1:
            rstd1, nmr1 = _adaln_stats(nc, n1_stat, lambda it: x_tiles[it],
                                       16, eps_sb)

        # ---------------- norm scale/shift params ----------------
        # emb = t @ nw + nb  -> [1, 1024] -> [128, 8] columns. Only layer 1
        # is needed early; layers 2/4 are deferred past the phase-2 issue so
        # their 2.1MB weight DMAs queue behind the attention weights.
        def _emb(l):
            with (
                tc.tile_pool(name=f"nwp{l}", bufs=1) as nwp,
                tc.tile_pool(name=f"embp{l}", bufs=1) as embp,
                tc.tile_pool(name=f"ps_emb{l}", bufs=2, space="PSUM") as ps_emb,
            ):
                nw_sb = nwp.tile([P, 4, 2 * D], bf16, tag="nw")
                nc.gpsimd.dma_start(
                    nw_sb, nw[l][:].rearrange("(k p) n -> p k n", p=P)
                )
                nb_row = embp.tile([1, 2 * D], f32, tag="nbrow")
                nc.sync.dma_start(nb_row, nb_[l][:].rearrange("(a n) -> a n", a=1))
                emb_ps = ps_emb.tile([1, 2 * D], f32, tag="embps")
                for half in range(2):
                    for kt in range(4):
                        nc.tensor.matmul(
                            emb_ps[:, half * D:(half + 1) * D],
                            tT[:, kt:kt + 1],
                            nw_sb[:, kt, half * D:(half + 1) * D],
                            start=(kt == 0), stop=(kt == 3),
                        )
                emb_row = embp.tile([1, 2 * D], f32, tag="embrow")
                nc.vector.tensor_tensor(emb_row, emb_ps, nb_row, op=OP.add)
                # row -> per-partition columns via tiny PE transposes;
                # scale columns (0:4) get the +1 fused into the psum copy
                ab_l = const.tile([P, 8], f32, tag=f"ab{l}")
                for col in range(8):
                    tp = ps_emb.tile([P, 1], f32, tag="embT")
                    nc.tensor.transpose(
                        tp, emb_row[0:1, col * P:(col + 1) * P],
                        ident_f32[0:1, 0:1]
                    )
                    nc.vector.tensor_scalar(
                        ab_l[:, col:col + 1], tp,
                        1.0 if col < 4 else 0.0, None, op0=OP.add,
                    )
            return ab_l

        ab = {1: _emb(1)}



        # ---------------- attention weights (bf16 via DMA cast) ----------
        # a1 stack shares addresses with ff_w1, a2 stack with ff_w2
        # (sequential lifetimes; Tile inserts the WAR deps).
        a_sb = {}
        for a, wtag in ((1, "wbig1"), (2, "wbig2")):
            stack = wpool.tile([P, 4, 4, D], bf16, tag=wtag)
            for wi, w in enumerate("qkvo"):
                nc.gpsimd.dma_start(
                    stack[:, :, wi, :],
                    aw[a, w][:].rearrange("(k p) n -> p k n", p=P),
                )
                a_sb[a, w] = stack[:, :, wi, :]
            ob = wpool.tile([1, D], bf16, tag=f"a{a}ob")
            nc.gpsimd.dma_start(ob, aw[a, "ob"][:].rearrange("(a n) -> a n", a=1))
            a_sb[a, "ob"] = ob


        # ---------------- phase 1: adaln1 apply -> h1T -------------------
        if max_phase < 1:
            final = own_x
        else:
            _adaln_apply(nc, tc, lambda it: x_tiles[it], 16, ab[1], rstd1,
                         nmr1, h1T, ident_bf16, "n1")
            final = own_x

        # ---------------- phase 2: projections k1T, vC1, q1z -------------
        if max_phase >= 2:
            k1T = act.tile([P, 4, NB], bf16, tag="tB")
            vC1 = _vc_tile(nc, act, "tC", 16)
            q1z = act.tile([P, 4, 2, ROWS], bf16, tag="tD")
            nc.vector.memset(q1z[HD:P, :, 0, :], 0.0)
            nc.vector.memset(q1z[0:HD, :, 1, :], 0.0)
            with tc.tile_pool(name="ps_proj1", bufs=4, space="PSUM") as ps_proj:
                for dt_ in range(4):
                    for jc in range(4):
                        ps = ps_proj.tile([P, 512], f32, tag="proj")
                        for kt in range(4):
                            nc.tensor.matmul(
                                ps,
                                a_sb[1, "k"][:, kt, dt_ * P:(dt_ + 1) * P],
                                h1T[:, kt, jc * 512:(jc + 1) * 512],
                                start=(kt == 0), stop=(kt == 3),
                            )
                        nc.vector.tensor_copy(
                            k1T[:, dt_, jc * 512:(jc + 1) * 512], ps
                        )
                for jt in range(16):
                    ps = ps_proj.tile([P, 512], f32, tag="proj")
                    for kt in range(4):
                        nc.tensor.matmul(
                            ps,
                            h1T[:, kt, jt * P:(jt + 1) * P],
                            a_sb[1, "v"][:, kt, :],
                            start=(kt == 0), stop=(kt == 3),
                        )
                    _vc_copy(nc, vC1, jt, ps)
                for dt_ in range(4):
                    ps = ps_proj.tile([P, 512], f32, tag="proj")
                    for kt in range(4):
                        nc.tensor.matmul(
                            ps,
                            a_sb[1, "q"][:, kt, dt_ * P:(dt_ + 1) * P],
                            h1T[:, kt, 0:ROWS],
                            start=(kt == 0), stop=(kt == 3),
                        )
                    nc.vector.tensor_copy(q1z[0:HD, dt_, 0, :], ps[0:HD, :])
                    nc.vector.tensor_copy(q1z[HD:P, dt_, 1, :], ps[HD:P, :])

        # ------- early cross-attn prep: condT, k2T, v2 (independent of x) ----
        if max_phase >= 4:
            condT = act.tile([P, 4, NCTX], bf16, tag="tE2")
            with (
                tc.tile_pool(name="cin", bufs=3) as cin,
                tc.tile_pool(name="ps_ct", bufs=2, space="PSUM") as ps_ct,
            ):
                for it in range(8):
                    c_sb = cin.tile([P, D], f32, tag="ctile")
                    nc.sync.dma_start(c_sb, condb[:][it * P:(it + 1) * P, :])
                    ct = ps_ct.tile([P, 4, P], f32, tag="ct")
                    for b in range(4):
                        nc.tensor.transpose(
                            ct[:, b, :], c_sb[:, b * P:(b + 1) * P], ident_f32
                        )
                    for b in range(4):
                        nc.vector.tensor_copy(
                            condT[:, b, it * P:(it + 1) * P], ct[:, b, :]
                        )
            k2T = act.tile([P, 4, NCTX], bf16, tag="tX")
            vC2 = _vc_tile(nc, act, "tI", 8)
            with tc.tile_pool(name="ps_proj2a", bufs=4, space="PSUM") as ps_proj:
                for dt_ in range(4):
                    for jc in range(2):
                        ps = ps_proj.tile([P, 512], f32, tag="proj")
                        for kt in range(4):
                            nc.tensor.matmul(
                                ps,
                                a_sb[2, "k"][:, kt, dt_ * P:(dt_ + 1) * P],
                                condT[:, kt, jc * 512:(jc + 1) * 512],
                                start=(kt == 0), stop=(kt == 3),
                            )
                        nc.vector.tensor_copy(
                            k2T[:, dt_, jc * 512:(jc + 1) * 512], ps
                        )
                for jt in range(8):
                    ps = ps_proj.tile([P, 512], f32, tag="proj")
                    for kt in range(4):
                        nc.tensor.matmul(
                            ps,
                            condT[:, kt, jt * P:(jt + 1) * P],
                            a_sb[2, "v"][:, kt, :],
                            start=(kt == 0), stop=(kt == 3),
                        )
                    _vc_copy(nc, vC2, jt, ps)

        # deferred adaln2/adaln3 params: weight DMAs queue behind the
        # attention stacks, PE work lands in the proj -> att1 seam
        ab[2] = _emb(2)
        ab[4] = _emb(4)

        # ---------------- phase 3: attention 1 ---------------------------
        if max_phase >= 3:
            x2 = act.tile([P, 4, D], f32, tag="tF")
            _attention(nc, tc, act, q1z, k1T, vC1, 16, a_sb[1, "o"],
                       a_sb[1, "ob"], ones_row, own_x, x2, "att1")
            final = x2

        # ---------------- phase 4: adaln2 + cross-attn -------------------
        if max_phase >= 4:
            h2T = act.tile([P, 4, ROWS], bf16, tag="tH")
            _adaln_to_hT(nc, tc, lambda it: x2[:, it, :], 4, ab[2], h2T,
                         ident_bf16, eps_sb, "n2")

            q2z = act.tile([P, 4, 2, ROWS], bf16, tag="tE2")
            nc.vector.memset(q2z[HD:P, :, 0, :], 0.0)
            nc.vector.memset(q2z[0:HD, :, 1, :], 0.0)
            with tc.tile_pool(name="ps_proj2b", bufs=2, space="PSUM") as ps_proj:
                for dt_ in range(4):
                    ps = ps_proj.tile([P, 512], f32, tag="proj")
                    for kt in range(4):
                        nc.tensor.matmul(
                            ps,
                            a_sb[2, "q"][:, kt, dt_ * P:(dt_ + 1) * P],
                            h2T[:, kt, :],
                            start=(kt == 0), stop=(kt == 3),
                        )
                    nc.vector.tensor_copy(q2z[0:HD, dt_, 0, :], ps[0:HD, :])
                    nc.vector.tensor_copy(q2z[HD:P, dt_, 1, :], ps[HD:P, :])

            x3 = act.tile([P, 4, D], f32, tag="tG")
            _attention(nc, tc, act, q2z, k2T, vC2, 8, a_sb[2, "o"],
                       a_sb[2, "ob"], ones_row, x2, x3, "att2")
            final = x3

        # ---------------- phase 5: adaln3 + GEGLU FFN --------------------
        if max_phase >= 5:
            h3T = act.tile([P, 4, ROWS], bf16, tag="tD")
            _adaln_to_hT(nc, tc, lambda it: x3[:, it, :], 4, ab[4], h3T,
                         ident_bf16, eps_sb, "n4")

            # ff_w1 halves live in the dead h1T / vC1 slots so their DMAs
            # start as soon as phase 2 / attention-1 stop reading those,
            # instead of waiting for the a1 weight stack to die.
            w1a = act.tile([P, 4, 4 * D], bf16, tag="tA")
            nc.gpsimd.dma_start(
                w1a, ff_w1[:][:, 0:4 * D].rearrange("(k p) n -> p k n", p=P))
            w1b = act.tile([P, 4, 4 * D], bf16, tag="tC")
            nc.gpsimd.dma_start(
                w1b, ff_w1[:][:, 4 * D:8 * D].rearrange("(k p) n -> p k n", p=P))
            w2_sb = wpool.tile([P, 16, D], bf16, tag="wbig2")
            nc.gpsimd.dma_start(w2_sb, ff_w2[:].rearrange("(k p) n -> p k n", p=P))
            b1_sb = const.tile([P, 32], f32)
            nc.sync.dma_start(b1_sb, ff_b1[:].rearrange("(k p) -> p k", p=P))
            b2_row = const.tile([1, D], bf16)
            nc.gpsimd.dma_start(b2_row, ff_b2[:].rearrange("(a n) -> a n", a=1))

            ugT = act.tile([P, 16, ROWS], bf16, tag="tB")
            with (
                tc.tile_pool(name="ps_z", bufs=4, space="PSUM") as ps_z,
                tc.tile_pool(name="gact", bufs=3) as gact_pool,
            ):
                for ut in range(16):
                    zu = ps_z.tile([P, ROWS], f32, tag="z")
                    zg = ps_z.tile([P, ROWS], f32, tag="z")
                    for kt in range(4):
                        nc.tensor.matmul(
                            zu, w1a[:, kt, ut * P:(ut + 1) * P],
                            h3T[:, kt, :], start=(kt == 0), stop=(kt == 3),
                        )
                    for kt in range(4):
                        nc.tensor.matmul(
                            zg, w1b[:, kt, ut * P:(ut + 1) * P],
                            h3T[:, kt, :], start=(kt == 0), stop=(kt == 3),
                        )
                    gact = gact_pool.tile([P, ROWS], bf16, tag="gact")
                    nc.scalar.activation(
                        gact, zg, AF.Gelu, bias=b1_sb[:, 16 + ut:17 + ut], scale=1.0
                    )
                    nc.vector.scalar_tensor_tensor(
                        ugT[:, ut, :], zu, b1_sb[:, ut:ut + 1], gact,
                        op0=OP.add, op1=OP.mult,
                    )

            out_sb = act.tile([P, 4, D], f32, tag="tC")
            with tc.tile_pool(name="ps_y", bufs=2, space="PSUM") as ps_y:
                for it in range(4):
                    ps = ps_y.tile([P, D], f32, tag="y")
                    for kt in range(16):
                        nc.tensor.matmul(
                            ps, ugT[:, kt, it * P:(it + 1) * P],
                            w2_sb[:, kt, :],
                            start=(kt == 0), stop=False,
                        )
                    nc.tensor.matmul(
                        ps, ones_row[0:1, 0:P], b2_row, start=False, stop=True,
                    )
                    nc.vector.tensor_tensor(
                        out_sb[:, it, :], ps, x3[:, it, :], op=OP.add
                    )
            final = out_sb

        for it_ in range(4):
            nc.sync.dma_start(out[:][it_ * P:(it_ + 1) * P, :], final[:, it_, :])

    nc.compile()
    return nc


def _shard_inputs(inputs):
    """Build the 8 per-core input maps."""
    x = np.ascontiguousarray(inputs["x"], dtype=np.float32)
    t = np.ascontiguousarray(inputs["t"], dtype=np.float32)
    cond = np.ascontiguousarray(inputs["cond"], dtype=np.float32)
    shared = {}
    for k in ("n1_w", "n1_b", "n2_w", "n2_b", "n4_w", "n4_b",
              "a1_q", "a1_k", "a1_v", "a1_o", "a1_ob",
              "a2_q", "a2_k", "a2_v", "a2_o", "a2_ob",
              "ff_w1", "ff_b1", "ff_w2", "ff_b2"):
        shared[k] = np.ascontiguousarray(inputs[k], dtype=np.float32)
    in_maps = []
    for c in range(NCORES):
        b = c // 4
        r0 = (c % 4) * ROWS
        m = dict(shared)
        m["xb"] = np.ascontiguousarray(np.roll(x[b], -r0, axis=0))
        m["condb"] = np.ascontiguousarray(cond[b])
        m["t"] = np.ascontiguousarray(t[b, 0])
        in_maps.append(m)
    return in_maps


def kernel(**inputs) -> np.ndarray:
    if "nc" not in _CACHED:
        _CACHED["nc"] = build()
    nc = _CACHED["nc"]
    in_maps = _shard_inputs(inputs)
    res = run_bass_kernel_spmd(nc, in_maps, core_ids=list(range(NCORES)))
    outs = [res.results[c]["out"] for c in range(NCORES)]
    full = np.concatenate(outs, axis=0).reshape(B, N, D)
    return full.astype(np.float32)

